# revision 1
# baseline (speedup 1.0000x reference)
"""Causal self-attention (GQA + rms_norm + RoPE) on 8 TRN2 NeuronCores.

Sharding: tensor-parallel over heads. Core c owns q-heads {2c, 2c+1} and
kv-head c//2 (GQA groups intact; each kv head is replicated on 2 cores).
Wo is sharded along its input (head) dim, so each core emits a partial
(T, C) output; the host sums the 8 partials.

Per-core dataflow, software-pipelined over 512-row blocks bq:
  stage A (rows 512*bq..512*bq+511): qkv = xT-tiles @ Wqkv (fp32r);
          rms_norm via tensor_tensor_reduce (sum of squares) + one batched
          Sqrt + reciprocal; RoPE fused with the rstd scaling via
          scalar_tensor_tensor (the sin table's first half is pre-negated
          host-side so rope is mul/mul/add); PE-transpose q,k into [d, t]
          bf16 tiles; v kept natural [s, d] bf16.
  stage B per head: over causal s-tiles of 128: S^T[s,tq] = kT_tile.T @
          qT_block (bf16); exp on ScalarE with no max-subtraction
          (rms_norm bounds |score*scale| <= sqrt(D) ~ 11.3, so exp is in
          range); causal mask on the 4 diagonal tiles via
          gpsimd.affine_select; PV (v_tile.T @ P^T) and a ones-column
          matmul accumulate y^T and softmax row-sums in PSUM.
  stage C: normalize y^T by 1/rowsum (K=1 broadcast matmul + DVE mul),
          out[t,:] = sum_h yT_h.T @ Wo_h, PSUM evacuation split between
          VectorE and ScalarE, DMA the partial to HBM.

All matmuls run at 1 PE cycle/row: fp32r for the C=2048-contraction QKV
(needs N>=256), bf16 for attention and Wo.
"""

import math

import ml_dtypes
import numpy as np

import concourse.bass as bass
import concourse.mybir as mybir
import concourse.tile as tile
from concourse import bacc
from concourse.bass_utils import run_bass_kernel_spmd
from concourse.masks import make_identity

F32 = mybir.dt.float32
F32R = mybir.dt.float32r
BF16 = mybir.dt.bfloat16
MUL = mybir.AluOpType.mult
ADD = mybir.AluOpType.add

C = 2048          # model dim
H, KV, D = 16, 4, 128
REP = H // KV
N_CORES = 8
HPC = H // N_CORES          # q heads per core (2)
QKV_N = HPC * D + 2 * D     # qkv output columns per core (512)
EPS = 1e-6
SCALE = 1.0 / math.sqrt(D)
import os
USE_PBCAST = os.environ.get("K_PBCAST", "1") == "1"
USE_DVE_RSQRT = os.environ.get("K_RSQRT", "1") == "1"
USE_NEG_SWAP = os.environ.get("K_SWAP", "1") == "1"
SKEW = int(os.environ.get("K_SKEW", "5"))
USE_PIPE = os.environ.get("K_PIPE", "1") == "1"


def build_nc(T: int) -> bass.Bass:
    assert T % 512 == 0
    n_tt = T // 128        # 128-row t-tiles
    n_blk = T // 512       # 512-wide tq blocks
    n_ct = C // 128        # contraction tiles for qkv

    nc = bacc.Bacc()
    xT_d = nc.dram_tensor("xT", [C, T], F32R, kind="ExternalInput")
    wqkv_d = nc.dram_tensor("wqkv", [C, QKV_N], F32R, kind="ExternalInput")
    wo_d = nc.dram_tensor("wo", [HPC * D, C], BF16, kind="ExternalInput")
    cs_d = nc.dram_tensor("cs", [T, 2 * D], F32, kind="ExternalInput")
    out_d = nc.dram_tensor("out", [T, C], F32, kind="ExternalOutput")

    xT_r = xT_d[:].rearrange("(ct p) t -> p ct t", p=128)      # [128, n_ct, T]
    wqkv_r = wqkv_d[:].rearrange("(ct p) n -> p ct n", p=128)  # [128, n_ct, 512]
    wo_r = wo_d[:].rearrange("(h p) n -> p h n", p=128)        # [128, HPC, C]
    cs_r = cs_d[:].rearrange("(n p) d -> n p d", p=128)        # [n_tt, 128, 256]
    out_r = out_d[:].rearrange("(n p) c -> n p c", p=128)      # [n_tt, 128, C]

    with tile.TileContext(nc) as tc:
        with (
            tc.tile_pool(name="singles", bufs=1) as singles,
            tc.tile_pool(name="xin", bufs=4) as xin,
            tc.tile_pool(name="csin", bufs=5) as csin,
            tc.tile_pool(name="t1", bufs=3) as t1,
            tc.tile_pool(name="rpp", bufs=24) as rpp,
            tc.tile_pool(name="abp", bufs=8) as abp,
            tc.tile_pool(name="pp", bufs=3) as pp,
            tc.tile_pool(name="ptp", bufs=10) as ptp,
            tc.tile_pool(name="small", bufs=4) as small,
            tc.tile_pool(name="ot", bufs=3) as otp,
            tc.tile_pool(name="ps", bufs=8, space="PSUM") as psp,
        ):
            # ---- constants / resident tensors ----
            ident = singles.tile([128, 128], F32)
            make_identity(nc, ident)
            ones_col = singles.tile([128, 1], BF16)
            nc.vector.memset(ones_col, 1.0)
            ones_row = singles.tile([1, 128], F32)
            nc.vector.memset(ones_row, 1.0)
            wqkv_s = singles.tile([128, n_ct, QKV_N], F32R)

            def load_wqkv(wc):
                sl = slice(wc * n_ct // 4, (wc + 1) * n_ct // 4)
                nc.sync.dma_start(out=wqkv_s[:, sl, :], in_=wqkv_r[:, sl, :])

            wo_s = singles.tile([128, HPC, C], BF16)

            qT = singles.tile([128, HPC, T], BF16)   # [d, h, t]
            kT = singles.tile([128, T], BF16)        # [d, s]
            v = singles.tile([128, n_tt, D], BF16)   # [s%128, s//128, d]
            yT = singles.tile([128, HPC, T], BF16)   # [d, h, t]

            rp_store = {}

            def stage_a_dma(it):
                """issue the input DMAs for t-tile it."""
                t0 = it * 128
                xt = xin.tile([128, n_ct, 128], F32R)
                nc.sync.dma_start(out=xt, in_=xT_r[:, :, t0:t0 + 128])
                cst = csin.tile([128, 2 * D], F32)
                nc.sync.dma_start(out=cst, in_=cs_r[it])
                return xt, cst

            def stage_a_mm(it, pre):
                """qkv + rms + rope for t-tile it (no transposes)."""
                t0 = it * 128
                xt, cst = pre
                cos = cst[:, 0:D]
                snv = cst[:, D:2 * D]     # [-sin[0:64] | sin[64:128]]

                ps = psp.tile([128, QKV_N], F32, tag="ps")
                for ct in range(n_ct):
                    nc.tensor.matmul(
                        ps, xt[:, ct, :], wqkv_s[:, ct, :],
                        start=(ct == 0), stop=(ct == n_ct - 1),
                    )
                # v: plain evacuation (cast to bf16)
                nc.vector.tensor_copy(v[:, it, :], ps[:, (HPC + 1) * D:(HPC + 2) * D])

                # rope first (rotation preserves row norms exactly), then
                # rms stats off the roped SBUF values -- walrus allows only
                # one PSUM operand per DVE instruction
                abs_ = []
                ssv = small.tile([128, HPC + 1], F32, tag="ssv")
                sq = t1.tile([128, D], F32, tag="sq")
                for j in range(HPC + 1):
                    chunk = ps[:, j * D:(j + 1) * D]
                    a = t1.tile([128, D], F32, tag="a")
                    nc.vector.tensor_mul(a, chunk, cos)
                    b = t1.tile([128, D], F32, tag="b")
                    if USE_NEG_SWAP:
                        # same chunk with the two D/2 halves swapped (rotate_half)
                        swapped = chunk.rearrange("p (two half) -> p two half", two=2)[:, ::-1, :]
                        nc.vector.tensor_mul(b, swapped, snv)
                    else:
                        nc.vector.tensor_mul(b[:, 0:D // 2], chunk[:, D // 2:D], snv[:, 0:D // 2])
                        nc.vector.tensor_mul(b[:, D // 2:D], chunk[:, 0:D // 2], snv[:, D // 2:D])
                    ab = abp.tile([128, D], F32, tag="ab")
                    nc.vector.tensor_add(ab, a, b)
                    abs_.append(ab)
                    # (tensor_tensor_reduce wedges the device -- use plain
                    # DVE square + free-axis reduce instead)
                    nc.vector.tensor_mul(sq, ab, ab)
                    nc.vector.reduce_sum(
                        ssv[:, j:j + 1], sq, axis=mybir.AxisListType.X,
                    )
                # ssv holds sum(ab^2); convert to mean + eps
                nc.vector.tensor_scalar(
                    out=ssv, in0=ssv, scalar1=1.0 / D, scalar2=EPS,
                    op0=MUL, op1=ADD,
                )
                # rstd = rsqrt(ssv)
                rstd = small.tile([128, HPC + 1], F32, tag="rstd")
                if USE_DVE_RSQRT:
                    # entirely on DVE (quake-style seed + two Newton steps)
                    # -- keeps ScalarE's function table on Exp
                    I32 = mybir.dt.int32
                    nc.vector.tensor_scalar(
                        out=rstd.bitcast(I32), in0=ssv.bitcast(I32),
                        scalar1=1, scalar2=None,
                        op0=mybir.AluOpType.logical_shift_right,
                    )
                    nc.vector.tensor_scalar(
                        out=rstd.bitcast(I32), in0=rstd.bitcast(I32),
                        scalar1=0x5F3759DF, scalar2=-1,
                        op0=mybir.AluOpType.subtract, op1=MUL,
                    )
                    mh = small.tile([128, HPC + 1], F32, tag="mh")
                    nc.vector.tensor_scalar(
                        out=mh, in0=ssv, scalar1=-0.5, scalar2=None, op0=MUL,
                    )
                    for _ in range(2):
                        u = small.tile([128, HPC + 1], F32, tag="u")
                        nc.vector.tensor_mul(u, rstd, rstd)
                        nc.vector.tensor_mul(u, u, mh)
                        nc.vector.tensor_scalar(
                            out=u, in0=u, scalar1=1.5, scalar2=None, op0=ADD,
                        )
                        nc.vector.tensor_mul(rstd, rstd, u)
                else:
                    nc.scalar.activation(
                        rstd, ssv, mybir.ActivationFunctionType.Sqrt,
                    )
                    nc.vector.reciprocal(rstd, rstd)
                rps = []
                for j in range(HPC + 1):
                    rp = rpp.tile([128, D], F32, tag="rp")
                    nc.vector.tensor_scalar_mul(rp, abs_[j], rstd[:, j:j + 1])
                    rps.append(rp)
                rp_store[it] = rps

            def stage_a_tp(it):
                """PE-transpose q,k of t-tile it into qT/kT."""
                t0 = it * 128
                rps = rp_store.pop(it)
                for j in range(HPC + 1):
                    tp = psp.tile([128, 128], F32, tag="ps")
                    nc.tensor.transpose(tp, rps[j], ident)
                    if j < HPC:
                        nc.vector.tensor_copy(qT[:, j, t0:t0 + 128], tp)
                    else:
                        nc.vector.tensor_copy(kT[:, t0:t0 + 128], tp)

            def stage_b(h, bq):
                """attention for (head h, tq block bq), skew-2 pipelined."""
                q0 = bq * 512
                nst = 4 * bq + 4   # causal s-tiles
                pv = psp.tile([128, 512], F32, tag="ps")
                rsum = psp.tile([1, 512], F32, tag="ps")
                pts = {}

                def emit_score(st):
                    s0 = st * 128
                    sp = psp.tile([128, 512], F32, tag="ps")
                    nc.tensor.matmul(
                        sp, kT[:, s0:s0 + 128], qT[:, h, q0:q0 + 512],
                        start=True, stop=True,
                    )
                    pt = ptp.tile([128, 512], BF16, tag="pt")
                    nc.scalar.activation(
                        pt, sp, mybir.ActivationFunctionType.Exp,
                        scale=SCALE,
                    )
                    if st >= 4 * bq:  # diagonal tile: causal mask
                        nc.gpsimd.affine_select(
                            out=pt, in_=pt,
                            compare_op=mybir.AluOpType.is_ge,
                            fill=0.0,
                            base=q0 - s0,
                            pattern=[[1, 512]],
                            channel_multiplier=-1,
                        )
                    pts[st] = pt

                def emit_consume(st):
                    pt = pts.pop(st)
                    nc.tensor.matmul(
                        pv, v[:, st, :], pt,
                        start=(st == 0), stop=(st == nst - 1),
                    )
                    nc.tensor.matmul(
                        rsum, ones_col, pt,
                        start=(st == 0), stop=(st == nst - 1),
                    )

                for st in range(nst):
                    emit_score(st)
                    if st >= SKEW:
                        emit_consume(st - SKEW)
                for st in range(max(0, nst - SKEW), nst):
                    emit_consume(st)

                rrs = small.tile([1, 512], F32, tag="rrs")
                nc.vector.reciprocal(rrs, rsum)
                if USE_PBCAST:
                    # broadcast 1/rowsum across partitions on gpsimd
                    bc = pp.tile([128, 512], F32, tag="bc")
                    nc.gpsimd.partition_broadcast(bc, rrs)
                    nc.vector.tensor_mul(yT[:, h, q0:q0 + 512], pv, bc)
                else:
                    bcp = psp.tile([128, 512], F32, tag="ps")
                    nc.tensor.matmul(
                        bcp, ones_row, rrs,
                        start=True, stop=True,
                    )
                    ytmp = pp.tile([128, 512], F32, tag="bc")
                    nc.scalar.copy(ytmp, pv)
                    nc.vector.tensor_mul(yT[:, h, q0:q0 + 512], ytmp, bcp)

            def stage_c(it):
                """out = yT.T @ Wo (partial) for t-tile it."""
                t0 = it * 128
                ot = otp.tile([128, C], F32)
                for nb in range(C // 512):
                    po = psp.tile([128, 512], F32, tag="ps")
                    for h in range(HPC):
                        nc.tensor.matmul(
                            po, yT[:, h, t0:t0 + 128],
                            wo_s[:, h, nb * 512:(nb + 1) * 512],
                            start=(h == 0), stop=(h == HPC - 1),
                        )
                    if nb % 2 == 0:
                        nc.vector.tensor_copy(ot[:, nb * 512:(nb + 1) * 512], po)
                    else:
                        nc.scalar.copy(ot[:, nb * 512:(nb + 1) * 512], po)
                if it >= n_tt - 2:
                    # tail: split the final stores so the end-of-kernel DMA
                    # drain works on smaller chunks
                    nc.sync.dma_start(out=out_r[it][:, 0:C // 2], in_=ot[:, 0:C // 2])
                    nc.sync.dma_start(out=out_r[it][:, C // 2:C], in_=ot[:, C // 2:C])
                else:
                    nc.sync.dma_start(out=out_r[it], in_=ot)

            # software pipeline over 512-row blocks. Emission order shapes
            # each engine's static FIFO: block bq+1's qkv matmuls and DVE
            # chains are issued, then bq's second-head attention (independent
            # PE work that hides the DVE chains), then the transposes
            # (whose inputs are ready by now), then the next block's first
            # head, then stage_c (hiding the yT normalize chain).
            if USE_PIPE:
                def emit_amm_group(b):
                    if b < n_blk:
                        for it in range(4 * b, 4 * b + 4):
                            stage_a_mm(it, stage_a_dma(it))

                def emit_atp_group(b):
                    if b < n_blk:
                        for it in range(4 * b, 4 * b + 4):
                            stage_a_tp(it)

                load_wqkv(0)
                pre0 = stage_a_dma(0)
                for wc in range(1, 4):
                    load_wqkv(wc)
                stage_a_mm(0, pre0)
                for it in range(1, 4):
                    stage_a_mm(it, stage_a_dma(it))
                nc.sync.dma_start(out=wo_s, in_=wo_r)
                emit_atp_group(0)
                emit_amm_group(1)
                stage_b(0, 0)
                for bq in range(n_blk):
                    emit_amm_group(bq + 2)
                    stage_b(1, bq)
                    emit_atp_group(bq + 1)
                    if bq + 1 < n_blk:
                        stage_b(0, bq + 1)
                    for it in range(4 * bq, 4 * bq + 4):
                        stage_c(it)
            else:
                for wc in range(4):
                    load_wqkv(wc)
                nc.sync.dma_start(out=wo_s, in_=wo_r)
                for bq in range(n_blk):
                    for it in range(4 * bq, 4 * bq + 4):
                        stage_a_mm(it, stage_a_dma(it))
                        stage_a_tp(it)
                    for h in range(HPC):
                        stage_b(h, bq)
                    for it in range(4 * bq, 4 * bq + 4):
                        stage_c(it)

    nc.compile()
    return nc


_NC_CACHE: dict[int, bass.Bass] = {}


def _get_nc(T: int) -> bass.Bass:
    if T not in _NC_CACHE:
        _NC_CACHE[T] = build_nc(T)
    return _NC_CACHE[T]


def _rope_tables(T: int) -> np.ndarray:
    """[T, 2D] table: [cos | sin-variant], snv = [-sin[:, :D/2] | sin[:, D/2:]]."""
    inv_freq = 1.0 / (10000.0 ** (np.arange(0, D, 2, dtype=np.float64) / D))
    t = np.arange(T, dtype=np.float64)
    freqs = np.outer(t, inv_freq)
    emb = np.concatenate([freqs, freqs], axis=-1)
    cos = np.cos(emb)
    sin = np.sin(emb)
    snv = np.concatenate([-sin[:, :D // 2], sin[:, D // 2:]], axis=-1)
    return np.concatenate([cos, snv], axis=-1).astype(np.float32)


def kernel(x, Wq, Wk, Wv, Wo):
    x = np.asarray(x, dtype=np.float32)
    Wq = np.asarray(Wq, dtype=np.float32)
    Wk = np.asarray(Wk, dtype=np.float32)
    Wv = np.asarray(Wv, dtype=np.float32)
    Wo = np.asarray(Wo, dtype=np.float32)
    B, T, _ = x.shape
    assert B == 1

    nc = _get_nc(T)
    xT = np.ascontiguousarray(x.reshape(T, C).T)
    cs = _rope_tables(T)

    in_maps = []
    for c in range(N_CORES):
        g = c // 2
        h0 = c * HPC
        wqkv = np.ascontiguousarray(np.concatenate(
            [
                Wq[:, h0 * D:(h0 + HPC) * D],
                Wk[:, g * D:(g + 1) * D],
                Wv[:, g * D:(g + 1) * D],
            ],
            axis=1,
        ))
        woc = np.ascontiguousarray(Wo[h0 * D:(h0 + HPC) * D, :]).astype(ml_dtypes.bfloat16)
        in_maps.append({"xT": xT, "wqkv": wqkv, "wo": woc, "cs": cs})

    res = run_bass_kernel_spmd(nc, in_maps, core_ids=list(range(N_CORES)))
    acc = np.zeros((T, C), dtype=np.float64)
    for r in res.results:
        acc += r["out"]
    return acc.astype(np.float32).reshape(B, T, C)



# revision 14
# speedup vs baseline: 1.1440x; 1.1440x over previous
"""Causal self-attention (GQA + rms_norm + RoPE) on 8 TRN2 NeuronCores.

Sharding: tensor-parallel over heads. Core c owns q-heads {2c, 2c+1} and
kv-head c//2 (GQA groups intact; each kv head is replicated on 2 cores).
Wo is sharded along its input (head) dim, so each core emits a partial
(T, C) output; the host sums the 8 partials.

Per-core dataflow, software-pipelined over 512-row blocks bq:
  stage A (rows 512*bq..512*bq+511): qkv = xT-tiles @ Wqkv (bf16);
          rms_norm via DVE square + free-axis reduce + one batched
          rsqrt; RoPE applied before the rms scaling (rotation preserves
          row norms); PE-transpose q,k (bf16, 1 cyc/row) into [d, t]
          tiles; v kept natural [s, d] bf16.
  stage B per head: over causal s-tiles of 128, diagonal tiles first:
          S^T[s,tq] = kT_tile.T @ qT_block (bf16) restricted to the
          causal column suffix; exp on ScalarE with no max-subtraction
          (rms_norm bounds |score*scale| <= sqrt(D) ~ 11.3); causal mask
          on the diagonal tiles via gpsimd.affine_select; PV accumulates
          y^T in PSUM; softmax row-sums come from tiny N<=4 matmuls per
          s-tile (pt_chunk.T @ ones-ish) into a [128,4] PSUM column
          block -- output-free-size costing makes these ~free, unlike a
          [1,512] ones-row matmul. A PSUM bank tolerates only ONE
          start=True per accumulation lifetime (a start wipes the whole
          bank), so the k==0 row-sum matmul uses rhs e0=[1|0|0|0] to
          initialize all four columns at once, and everything after
          accumulates with start=False.
  stage B normalize: cast sums to bf16, move them to a [1,512] PSUM row
          with plain matmuls against R=[I|0|0|0] / identity (again one
          start), reciprocal into SBUF, gpsimd partition_broadcast, DVE
          multiply into yT.
  stage C: out[t,:] = sum_h yT_h.T @ Wo_h, PSUM evacuation split between
          VectorE and ScalarE into a bf16 tile, DMA the partial to HBM.

All matmuls bf16 (1 PE cycle per output row). IO in bf16 (x, Wqkv, Wo,
out partials) to halve DMA traffic; rope tables stay f32.
"""

import math

import ml_dtypes
import numpy as np

import concourse.bass as bass
import concourse.mybir as mybir
import concourse.tile as tile
from concourse import bacc
from concourse.bass_utils import run_bass_kernel_spmd
from concourse.masks import make_identity

F32 = mybir.dt.float32
BF16 = mybir.dt.bfloat16
MUL = mybir.AluOpType.mult
ADD = mybir.AluOpType.add

C = 2048          # model dim
H, KV, D = 16, 4, 128
REP = H // KV
N_CORES = 8
HPC = H // N_CORES          # q heads per core (2)
QKV_N = HPC * D + 2 * D     # qkv output columns per core (512)
EPS = 1e-6
SCALE = 1.0 / math.sqrt(D)
SKEW = 5


def build_nc(T: int) -> bass.Bass:
    assert T % 512 == 0
    n_tt = T // 128        # 128-row t-tiles
    n_blk = T // 512       # 512-wide tq blocks
    n_ct = C // 128        # contraction tiles for qkv

    nc = bacc.Bacc()
    xT_d = nc.dram_tensor("xT", [C, T], BF16, kind="ExternalInput")
    wqkv_d = nc.dram_tensor("wqkv", [C, QKV_N], BF16, kind="ExternalInput")
    wo_d = nc.dram_tensor("wo", [HPC * D, C], BF16, kind="ExternalInput")
    cs_d = nc.dram_tensor("cs", [T, 2 * D], F32, kind="ExternalInput")
    out_d = nc.dram_tensor("out", [T, C], BF16, kind="ExternalOutput")

    xT_r = xT_d[:].rearrange("(ct p) t -> p ct t", p=128)      # [128, n_ct, T]
    wqkv_r = wqkv_d[:].rearrange("(ct p) n -> p ct n", p=128)  # [128, n_ct, 512]
    wo_r = wo_d[:].rearrange("(h p) n -> p h n", p=128)        # [128, HPC, C]
    cs_r = cs_d[:].rearrange("(n p) d -> n p d", p=128)        # [n_tt, 128, 256]
    out_r = out_d[:].rearrange("(n p) c -> n p c", p=128)      # [n_tt, 128, C]

    with tile.TileContext(nc) as tc:
        with (
            tc.tile_pool(name="singles", bufs=1) as singles,
            tc.tile_pool(name="xin", bufs=5) as xin,
            tc.tile_pool(name="csin", bufs=5) as csin,
            tc.tile_pool(name="t1", bufs=3) as t1,
            tc.tile_pool(name="rpp", bufs=24) as rpp,
            tc.tile_pool(name="abp", bufs=8) as abp,
            tc.tile_pool(name="pp", bufs=6) as pp,
            tc.tile_pool(name="ptp", bufs=10) as ptp,
            tc.tile_pool(name="small", bufs=4) as small,
            tc.tile_pool(name="ot", bufs=3) as otp,
            tc.tile_pool(name="ps", bufs=8, space="PSUM") as psp,
        ):
            # ---- constants / resident tensors ----
            ident = singles.tile([128, 128], BF16)
            make_identity(nc, ident)
            ones_col = singles.tile([128, 1], BF16)
            nc.vector.memset(ones_col, 1.0)
            # e0: [1|0|0|0] columns; rmat: [I|0|0|0] -- single-start inits
            e0 = singles.tile([128, 4], BF16)
            nc.vector.memset(e0, 0.0)
            nc.vector.memset(e0[:, 0:1], 1.0)
            rmat = singles.tile([128, 512], BF16)
            nc.vector.memset(rmat[:, 128:512], 0.0)
            nc.vector.tensor_copy(rmat[:, 0:128], ident)
            wqkv_s = singles.tile([128, n_ct, QKV_N], BF16)

            def load_wqkv(wc):
                sl = slice(wc * n_ct // 4, (wc + 1) * n_ct // 4)
                nc.sync.dma_start(out=wqkv_s[:, sl, :], in_=wqkv_r[:, sl, :])

            wo_s = singles.tile([128, HPC, C], BF16)

            qT = singles.tile([128, HPC, T], BF16)   # [d, h, t]
            kT = singles.tile([128, T], BF16)        # [d, s]
            v = singles.tile([128, n_tt, D], BF16)   # [s%128, s//128, d]
            yT = singles.tile([128, HPC, T], BF16)   # [d, h, t]

            rp_store = {}
            xpair = {}

            def stage_a_dma(it):
                """issue the input DMAs for t-tile it (x fetched in pairs so
                the bf16 DMA keeps a 512B innermost run)."""
                if it % 2 == 0:
                    t0 = it * 128
                    xt = xin.tile([128, n_ct, 256], BF16)
                    nc.sync.dma_start(out=xt, in_=xT_r[:, :, t0:t0 + 256])
                    xpair[it // 2] = xt
                cst = csin.tile([128, 2 * D], F32)
                nc.sync.dma_start(out=cst, in_=cs_r[it])
                return cst

            def stage_a_mm(it, cst):
                """qkv + rms + rope for t-tile it (no transposes)."""
                xt = xpair[it // 2] if it % 2 == 0 else xpair.pop(it // 2)
                xs = xt[:, :, (it % 2) * 128:(it % 2) * 128 + 128]
                cos = cst[:, 0:D]
                snv = cst[:, D:2 * D]     # [-sin[0:64] | sin[64:128]]

                ps = psp.tile([128, QKV_N], F32, tag="ps")
                for ct in range(n_ct):
                    nc.tensor.matmul(
                        ps, xs[:, ct, :], wqkv_s[:, ct, :],
                        start=(ct == 0), stop=(ct == n_ct - 1),
                    )
                # v: plain evacuation (cast to bf16)
                nc.vector.tensor_copy(v[:, it, :], ps[:, (HPC + 1) * D:(HPC + 2) * D])

                # rope first (rotation preserves row norms exactly), then
                # rms stats off the roped SBUF values -- walrus allows only
                # one PSUM operand per DVE instruction
                abs_ = []
                ssv = small.tile([128, HPC + 1], F32, tag="ssv")
                sq = t1.tile([128, D], F32, tag="sq")
                for j in range(HPC + 1):
                    chunk = ps[:, j * D:(j + 1) * D]
                    a = t1.tile([128, D], F32, tag="a")
                    nc.vector.tensor_mul(a, chunk, cos)
                    b = t1.tile([128, D], F32, tag="b")
                    # same chunk with the two D/2 halves swapped (rotate_half)
                    swapped = chunk.rearrange("p (two half) -> p two half", two=2)[:, ::-1, :]
                    nc.vector.tensor_mul(b, swapped, snv)
                    ab = abp.tile([128, D], F32, tag="ab")
                    nc.vector.tensor_add(ab, a, b)
                    abs_.append(ab)
                    nc.vector.tensor_mul(sq, ab, ab)
                    nc.vector.reduce_sum(
                        ssv[:, j:j + 1], sq, axis=mybir.AxisListType.X,
                    )
                # ssv holds sum(ab^2); convert to mean + eps
                nc.vector.tensor_scalar(
                    out=ssv, in0=ssv, scalar1=1.0 / D, scalar2=EPS,
                    op0=MUL, op1=ADD,
                )
                # rstd = rsqrt(ssv), entirely on DVE (quake-style seed + two
                # Newton steps) -- keeps ScalarE's function table on Exp
                rstd = small.tile([128, HPC + 1], F32, tag="rstd")
                I32 = mybir.dt.int32
                nc.vector.tensor_scalar(
                    out=rstd.bitcast(I32), in0=ssv.bitcast(I32),
                    scalar1=1, scalar2=None,
                    op0=mybir.AluOpType.logical_shift_right,
                )
                nc.vector.tensor_scalar(
                    out=rstd.bitcast(I32), in0=rstd.bitcast(I32),
                    scalar1=0x5F3759DF, scalar2=-1,
                    op0=mybir.AluOpType.subtract, op1=MUL,
                )
                mh = small.tile([128, HPC + 1], F32, tag="mh")
                nc.vector.tensor_scalar(
                    out=mh, in0=ssv, scalar1=-0.5, scalar2=None, op0=MUL,
                )
                for _ in range(2):
                    u = small.tile([128, HPC + 1], F32, tag="u")
                    nc.vector.tensor_mul(u, rstd, rstd)
                    nc.vector.tensor_mul(u, u, mh)
                    nc.vector.tensor_scalar(
                        out=u, in0=u, scalar1=1.5, scalar2=None, op0=ADD,
                    )
                    nc.vector.tensor_mul(rstd, rstd, u)
                rps = []
                for j in range(HPC + 1):
                    rp = rpp.tile([128, D], BF16, tag="rp")
                    nc.vector.tensor_scalar_mul(rp, abs_[j], rstd[:, j:j + 1])
                    rps.append(rp)
                rp_store[it] = rps

            def stage_a_tp(it):
                """PE-transpose q,k of t-tile it into qT/kT (bf16, 1cyc/row)."""
                t0 = it * 128
                rps = rp_store.pop(it)
                for j in range(HPC + 1):
                    tp = psp.tile([128, 128], BF16, tag="ps")
                    nc.tensor.transpose(tp, rps[j], ident)
                    if j < HPC:
                        nc.vector.tensor_copy(qT[:, j, t0:t0 + 128], tp)
                    else:
                        nc.vector.tensor_copy(kT[:, t0:t0 + 128], tp)

            def stage_b(h, bq):
                """attention for (head h, tq block bq), skew pipelined.

                s-tiles are emitted diagonal-block first so the first matmul
                of each PSUM bank covers the full 512 columns (the only
                start=True) while later diagonal tiles only touch their
                causal column suffix. For bq == 0 the diagonal tiles run at
                full width so masking still covers every emitted column.
                """
                q0 = bq * 512
                order = [4 * bq + i for i in range(4)] + list(range(4 * bq))
                nst = len(order)
                pv = psp.tile([128, 512], F32, tag="ps")
                aux = psp.tile([128, 512], F32, tag="ps")
                rsT = aux[:, 0:4]          # per-chunk softmax row-sums
                pts = {}

                def emit_score(k):
                    st = order[k]
                    s0 = st * 128
                    w0 = 128 * k if (k < 4 and bq > 0) else 0
                    sp = psp.tile([128, 512], F32, tag="ps")
                    nc.tensor.matmul(
                        sp[:, w0:512], kT[:, s0:s0 + 128],
                        qT[:, h, q0 + w0:q0 + 512],
                        start=True, stop=True,
                    )
                    pt = ptp.tile([128, 512], BF16, tag="pt")
                    nc.scalar.activation(
                        pt[:, w0:512], sp[:, w0:512],
                        mybir.ActivationFunctionType.Exp,
                        scale=SCALE,
                    )
                    if k < 4:  # diagonal tile: causal mask
                        nc.gpsimd.affine_select(
                            out=pt[:, w0:512], in_=pt[:, w0:512],
                            compare_op=mybir.AluOpType.is_ge,
                            fill=0.0,
                            base=(q0 + w0) - s0,
                            pattern=[[1, 512 - w0]],
                            channel_multiplier=-1,
                        )
                    pts[k] = (pt, w0)

                def emit_consume(k):
                    st = order[k]
                    pt, w0 = pts.pop(k)
                    nc.tensor.matmul(
                        pv[:, w0:512], v[:, st, :], pt[:, w0:512],
                        start=(k == 0), stop=(k == nst - 1),
                    )
                    if k == 0:
                        # single start covering all 4 sum columns
                        nc.tensor.matmul(
                            rsT, pt[:, 0:128], e0,
                            start=True, stop=False, skip_group_check=True,
                        )
                        rest = range(1, 4)
                    else:
                        rest = range(w0 // 128, 4)
                    for j in rest:
                        nc.tensor.matmul(
                            rsT[:, j:j + 1], pt[:, 128 * j:128 * j + 128],
                            ones_col,
                            start=False,
                            stop=(k == nst - 1 and j == 3),
                            skip_group_check=True,
                        )

                for k in range(nst):
                    emit_score(k)
                    if k >= SKEW:
                        emit_consume(k - SKEW)
                for k in range(max(0, nst - SKEW), nst):
                    emit_consume(k)

                # normalize: move the sums to a [1,512] PSUM row via plain
                # matmuls (single start via rmat), then one reciprocal,
                # broadcast across partitions, multiply into yT.
                csum = small.tile([128, 4], BF16, tag="csum")
                nc.vector.tensor_copy(csum, rsT)
                row = aux[0:1, 0:512]      # rsT is dead once csum is read
                nc.tensor.matmul(
                    row, csum[:, 0:1], rmat,
                    start=True, stop=False, skip_group_check=True,
                )
                for j in range(1, 4):
                    nc.tensor.matmul(
                        aux[0:1, 128 * j:128 * j + 128], csum[:, j:j + 1],
                        ident,
                        start=False, stop=(j == 3), skip_group_check=True,
                    )
                rrs = small.tile([1, 512], F32, tag="rrs")
                nc.vector.reciprocal(rrs, row)
                bc = pp.tile([128, 512], F32, tag="bc")
                nc.gpsimd.partition_broadcast(bc, rrs)
                nc.vector.tensor_mul(yT[:, h, q0:q0 + 512], pv, bc)

            def stage_c(it):
                """out = yT.T @ Wo (partial, bf16) for t-tile it."""
                t0 = it * 128
                ot = otp.tile([128, C], BF16)
                for nb in range(C // 512):
                    po = psp.tile([128, 512], F32, tag="ps")
                    for h in range(HPC):
                        nc.tensor.matmul(
                            po, yT[:, h, t0:t0 + 128],
                            wo_s[:, h, nb * 512:(nb + 1) * 512],
                            start=(h == 0), stop=(h == HPC - 1),
                        )
                    if nb % 2 == 0:
                        nc.vector.tensor_copy(ot[:, nb * 512:(nb + 1) * 512], po)
                    else:
                        nc.scalar.copy(ot[:, nb * 512:(nb + 1) * 512], po)
                if it >= n_tt - 2:
                    # tail: split the final stores so the end-of-kernel DMA
                    # drain works on smaller chunks
                    nc.sync.dma_start(out=out_r[it][:, 0:C // 2], in_=ot[:, 0:C // 2])
                    nc.sync.dma_start(out=out_r[it][:, C // 2:C], in_=ot[:, C // 2:C])
                else:
                    nc.sync.dma_start(out=out_r[it], in_=ot)

            # software pipeline over 512-row blocks. Emission order shapes
            # each engine's static FIFO: block bq+1's qkv matmuls and DVE
            # chains are issued, then bq's second-head attention (independent
            # PE work that hides the DVE chains), then the transposes
            # (whose inputs are ready by now), then the next block's first
            # head, then stage_c (hiding the yT normalize chain).
            def emit_amm_group(b):
                if b < n_blk:
                    for it in range(4 * b, 4 * b + 4):
                        stage_a_mm(it, stage_a_dma(it))

            def emit_atp_group(b):
                if b < n_blk:
                    for it in range(4 * b, 4 * b + 4):
                        stage_a_tp(it)

            load_wqkv(0)
            pre0 = stage_a_dma(0)
            for wc in range(1, 4):
                load_wqkv(wc)
            stage_a_mm(0, pre0)
            for it in range(1, 4):
                stage_a_mm(it, stage_a_dma(it))
            nc.sync.dma_start(out=wo_s, in_=wo_r)
            emit_atp_group(0)
            emit_amm_group(1)
            stage_b(0, 0)
            for bq in range(n_blk):
                emit_amm_group(bq + 2)
                stage_b(1, bq)
                emit_atp_group(bq + 1)
                if bq + 1 < n_blk:
                    stage_b(0, bq + 1)
                for it in range(4 * bq, 4 * bq + 4):
                    stage_c(it)

    nc.compile()
    return nc


_NC_CACHE: dict[int, bass.Bass] = {}


def _get_nc(T: int) -> bass.Bass:
    if T not in _NC_CACHE:
        _NC_CACHE[T] = build_nc(T)
    return _NC_CACHE[T]


def _rope_tables(T: int) -> np.ndarray:
    """[T, 2D] table: [cos | sin-variant], snv = [-sin[:, :D/2] | sin[:, D/2:]]."""
    inv_freq = 1.0 / (10000.0 ** (np.arange(0, D, 2, dtype=np.float64) / D))
    t = np.arange(T, dtype=np.float64)
    freqs = np.outer(t, inv_freq)
    emb = np.concatenate([freqs, freqs], axis=-1)
    cos = np.cos(emb)
    sin = np.sin(emb)
    snv = np.concatenate([-sin[:, :D // 2], sin[:, D // 2:]], axis=-1)
    return np.concatenate([cos, snv], axis=-1).astype(np.float32)


def _in_maps(x, Wq, Wk, Wv, Wo):
    """Per-core input dicts (host-side sharding + dtype casts)."""
    B, T, _ = x.shape
    xT = np.ascontiguousarray(x.reshape(T, C).T).astype(ml_dtypes.bfloat16)
    cs = _rope_tables(T)
    maps = []
    for c in range(N_CORES):
        g = c // 2
        h0 = c * HPC
        wqkv = np.ascontiguousarray(np.concatenate(
            [
                Wq[:, h0 * D:(h0 + HPC) * D],
                Wk[:, g * D:(g + 1) * D],
                Wv[:, g * D:(g + 1) * D],
            ],
            axis=1,
        )).astype(ml_dtypes.bfloat16)
        woc = np.ascontiguousarray(Wo[h0 * D:(h0 + HPC) * D, :]).astype(ml_dtypes.bfloat16)
        maps.append({"xT": xT, "wqkv": wqkv, "wo": woc, "cs": cs})
    return maps


def kernel(x, Wq, Wk, Wv, Wo):
    x = np.asarray(x, dtype=np.float32)
    Wq = np.asarray(Wq, dtype=np.float32)
    Wk = np.asarray(Wk, dtype=np.float32)
    Wv = np.asarray(Wv, dtype=np.float32)
    Wo = np.asarray(Wo, dtype=np.float32)
    B, T, _ = x.shape
    assert B == 1

    nc = _get_nc(T)
    res = run_bass_kernel_spmd(
        nc, _in_maps(x, Wq, Wk, Wv, Wo), core_ids=list(range(N_CORES)),
    )
    acc = np.zeros((T, C), dtype=np.float32)
    for r in res.results:
        acc += np.asarray(r["out"], dtype=np.float32)
    return acc.reshape(B, T, C)
